# revision 39
# baseline (speedup 1.0000x reference)
"""Trainium2 Bass kernel for nn_Conv2d_24833500905755 (3x3 conv, B=32,
C_in=64, C_out=128, 56x56, pad 1, with the reference's mismatched
weight-flatten order).

Math: out[b,co,h,w] = sum_{c,di,dj} xpad[b,c,h+di,w+dj] * Wt[c,di*3+dj,co]
with Wt = K.reshape(576, C_OUT).reshape(C_IN, 9, C_OUT).

Data-parallel: 4 images per NeuronCore, 2 images packed on the
128-partition dim (fp16 matmuls, K=64 contraction per half, concurrent
PE row-group tiles). Raw-bass hand-scheduled engine programs.

Final design (~39.7us measured vs 40.6us session baseline; exec window
sometimes includes ~6.5us of NEFF init -> 45-46us readings, pure
profiler luck):
  - BLOCKLESS: all engine programs emitted into the main BB; cross-
    engine order is fully semaphore-enforced, so nc.Block()'s entry
    branches + exit barrier are skipped.
  - Input DMAs issued first, split over the two HWDGE rings: sync ring
    [p0, p1, p2] (x pair-0 pieces), scalar ring [W, q0, q1, q2]. Ring
    FIFO puts p0/W at the heads; SDMA round-robins between rings at
    packet granularity so both land by ~11.3us. MEASURED DEAD ENDS:
    all-inputs-on-one-ring (8 serial 0.7us desc-gens -> stream stalls,
    +6us); W split into tap pieces across rings (starves behind p1/p2
    in round-robin, W-tail at 13.4us, +4us); tile_position weight
    sharing to halve W (walrus codegen rejects cross-partition LDW);
    p1/p2 via gpsimd SWDGE to unshare early bandwidth (+1us, noisier).
  - Junk warm-up bridge (NJUNK=22 junk MMs, alternating PE row-group
    halves) covers block entry (~7.3us) to W arrival: HAM lifts the PE
    clock 1.2->2.4GHz at a free-running 3.4us window boundary after
    sustained activity; ANY pre-warm PE-idle gap defers the lift ~2us,
    so the bridge must overshoot W slightly.
  - Scalar's out-DMA desc-gen gated on its own copy COMPLETION (s_cp2):
    the sequencer's DIRECT2D otherwise runs 0.6us ahead of the ACT
    unit and a fast SDMA pickup reads half-written staging
    (intermittent 1e-2 corruption observed without the gate).
  - Full final wait on s_out KEPT. Dropping it measures ~1.9us faster
    and usually passes, but in-flight DMAs at NEFF teardown can wedge a
    core's queue across NEFF reloads into PERSISTENT garbage outputs
    (rel err 0.43, sticky until the queue recovers) -- seen when
    different kernel builds run back-to-back. Correctness first.
Fixed costs (verified untouchable): ~6.5us NEFF init (NRT sem-clear
wait + iram loads + barriers), ~0.9us entry preamble, ~6.5us runtime
epilogue (per-engine sem-op storm; NOT in the NEFF engine programs),
~0.7us/DMA HWDGE desc-gen, ~0.8us SDMA first-byte, ~0.4us sem receipt.
Stream itself runs at the fp16 MAC floor (pair of concurrent K=64
row-group matmuls = 448 cols / 2.4GHz =~ 190ns; 126 pairs =~ 24us).
"""

from contextlib import ExitStack

import numpy as np

import concourse.bass as bass
import concourse.mybir as mybir
from concourse.bass_utils import run_bass_kernel_spmd

B, C_IN, C_OUT, H = 32, 64, 128, 56
KS = 3
N_CORES = 8
BPC = B // N_CORES
HP = H + 2
MM_DT = mybir.dt.float16
NJUNK = 20                    # warm-up bridge matmuls (11 concurrent pairs):
                              # bridge ends ~11.4us =~ W + p0 arrival. Too
                              # short leaves a PE-idle gap before W lands,
                              # which resets the HAM warm-up window (~2us
                              # penalty); too long wastes ~0.37us/pair.
JR = 8                        # full-width junk (N=448), ALTERNATING row-group
                              # halves: HAM only registers "busy" (and lifts
                              # the PE clock gate) when both 64-row groups are
                              # streaming, i.e. full-array activity

# per-pair chunk lists: (start_row, n_rows). NOTE: a 4+4 split of the
# final chunk was tried twice (with and without the final s_out wait)
# and loses ~0.5us both ways: scalar's per-chunk copy+gate+desc chain
# (~1.2us) outlasts a 4-row chunk's MM span (~0.86us), so the sub-chunks
# serialize on scalar and the final DMA lands later, not earlier.
CHUNKS = [
    [(i * 8, 8) for i in range(7)],
    [(i * 8, 8) for i in range(7)],
]
NCH = [len(c) for c in CHUNKS]
CHUNK_OF = [(p, ci) for p in range(2) for ci in range(NCH[p])]
NCHT = len(CHUNK_OF)          # 14 global chunks


def build_nc(mm_dt=MM_DT, njunk=NJUNK):
    f32 = mybir.dt.float32
    nc = bass.Bass()
    x_ext = nc.declare_dram_parameter("x", [BPC, C_IN, HP, HP], mm_dt, isOutput=False)
    w_ext = nc.declare_dram_parameter("w", [2 * C_IN, KS * KS, C_OUT], mm_dt, isOutput=False)
    out_ext = nc.declare_dram_parameter("out", [BPC, C_OUT, H, H], mm_dt, isOutput=True)

    n_out_dmas = 2 * NCHT  # halves * chunks

    with ExitStack() as ctx:
        wt = ctx.enter_context(nc.sbuf_tensor("wt", [2 * C_IN, KS * KS, C_OUT], mm_dt))
        xps = [
            ctx.enter_context(nc.sbuf_tensor(f"xp{p}", [2 * C_IN, HP, HP], mm_dt))
            for p in range(2)
        ]
        # obs[half][chunk] - per-chunk fp16 staging (global chunk index)
        obs = [
            [
                ctx.enter_context(
                    nc.sbuf_tensor(
                        f"ob_{h}_{c}", [C_OUT, CHUNKS[p][ci][1], H], mm_dt
                    )
                )
                for c, (p, ci) in enumerate(CHUNK_OF)
            ]
            for h in range(2)
        ]
        actp = ctx.enter_context(nc.sbuf_tensor("actp", [C_OUT, 1], f32))
        # banks[slot][half] - 8 PSUM banks
        banks = [
            [
                ctx.enter_context(
                    nc.psum_tensor(f"ps_{s}_{h}", [C_OUT, 8, H], f32)
                )
                for h in range(2)
            ]
            for s in range(4)
        ]
        s_w = ctx.enter_context(nc.semaphore("s_w"))
        s_x = [ctx.enter_context(nc.semaphore(f"s_x{p}")) for p in range(2)]
        s_mm = ctx.enter_context(nc.semaphore("s_mm"))
        s_cp = ctx.enter_context(nc.semaphore("s_cp"))
        s_cp2 = ctx.enter_context(nc.semaphore("s_cp2"))
        s_out = ctx.enter_context(nc.semaphore("s_out"))

        src0 = x_ext[0:2].rearrange("b c h w -> (b c) h w")
        src1 = x_ext[2:4].rearrange("b c h w -> (b c) h w")
        # Two-ring split, measured best: desc-gen runs 2-wide (one DIRECT2D
        # =~0.7us of NX time per DMA) and W/p0 sit at each ring's head.
        # (All-on-one-ring serializes 8 desc-gens -> pieces land late and
        # the stream stalls; W split across pieces starves behind p1/p2 in
        # the packet round-robin -- both measured slower.)
        # p0 ALONE on the sync ring: its desc-gen and transfer stay early,
        # and once it completes (~9.7us) the sync ring runs dry, giving W
        # FULL SDMA bandwidth instead of round-robin sharing with p1/p2
        # (which aren't needed until 13us+ and queue behind W here).
        nc.sync.dma_start(out=xps[0][:, 0:10, :], in_=src0[:, 0:10, :]).then_inc(s_x[0], 16)
        nc.scalar.dma_start(out=wt[:], in_=w_ext[:]).then_inc(s_w, 16)
        nc.scalar.dma_start(out=xps[0][:, 10:34, :], in_=src0[:, 10:34, :]).then_inc(s_x[0], 16)
        nc.scalar.dma_start(out=xps[1][:, 0:10, :], in_=src1[:, 0:10, :]).then_inc(s_x[1], 16)
        nc.scalar.dma_start(out=xps[0][:, 34:HP, :], in_=src0[:, 34:HP, :]).then_inc(s_x[0], 16)
        nc.scalar.dma_start(out=xps[1][:, 10:34, :], in_=src1[:, 10:34, :]).then_inc(s_x[1], 16)
        nc.scalar.dma_start(out=xps[1][:, 34:HP, :], in_=src1[:, 34:HP, :]).then_inc(s_x[1], 16)

        # Warm-up bridge: junk matmuls on not-yet-loaded SBUF keep the PE's
        # HAM activity window hot while the input DMAs land. banks[3] is
        # first reused by chunk 3 (start=True clears it), well after these
        # complete.
        for wi in range(njunk):
            h = wi % 2
            c0 = h * C_IN
            nc.tensor.matmul(
                out=banks[3][h][:, 0:JR, :],
                lhsT=wt[c0 : c0 + C_IN, 0, :],
                rhs=xps[0][c0 : c0 + C_IN, 0:JR, 0:H],
                start=True,
                stop=True,
            )

        # Blockless: every engine program is emitted straight into the main
        # BB (like the pre-block DMAs above). Per-engine instruction order =
        # issue order, and cross-engine ordering is already fully semaphore-
        # enforced, so the Block's entry branches and exit barrier (~0.5-1us)
        # are pure overhead.
        sync, scalar, tensor, vector = nc.sync, nc.scalar, nc.tensor, nc.vector

        # scalar: trigger the ACT-table load now; first PSUM copy needs the
        # table at ~14us.
        scalar.copy(out=actp[:], in_=actp[:])

        tensor.wait_ge(s_w, 16)
        for c, (p, ci) in enumerate(CHUNK_OF):
            h0, rows = CHUNKS[p][ci]
            if ci == 0:
                tensor.wait_ge(s_x[p], 16)  # rows [0,10)
            elif ci == 1:
                tensor.wait_ge(s_x[p], 32)  # rows [10,34)
            elif ci == 4:
                tensor.wait_ge(s_x[p], 48)  # rows [34,58)
            if c >= 4:
                # WAR: bank slot c%4 last used by chunk c-4
                tensor.wait_ge(s_cp, c - 3)
                tensor.wait_ge(s_cp2, c - 3)
            for k in range(KS * KS):
                di, dj = divmod(k, KS)
                last = k == KS * KS - 1
                for half in range(2):
                    c0 = half * C_IN
                    mm = tensor.matmul(
                        out=banks[c % 4][half][:, 0:rows, :],
                        lhsT=wt[c0 : c0 + C_IN, k, :],
                        rhs=xps[p][
                            c0 : c0 + C_IN,
                            h0 + di : h0 + di + rows,
                            dj : dj + H,
                        ],
                        start=(k == 0),
                        stop=last,
                    )
                    if last and half == 1:
                        mm.then_inc(s_mm, 2)

            # vector: half-0 PSUM->SBUF fp32->fp16 copy
            vector.wait_ge(s_mm, 2 * (c + 1))
            vector.tensor_copy(
                out=obs[0][c][:],
                in_=banks[c % 4][0][:, 0:rows, :],
            ).then_inc(s_cp, 1)

            # sync: half-0 output DMA (gated on vector copy completion)
            dst0 = out_ext[2 * p : 2 * p + 1].rearrange("b c h w -> (b c) h w")
            sync.wait_ge(s_cp, c + 1)
            sync.dma_start(
                out=dst0[:, h0 : h0 + rows, :], in_=obs[0][c][:]
            ).then_inc(s_out, 16)

            # scalar: half-1 copy + output DMA. Desc-gen gated on copy
            # COMPLETION: the sequencer's DIRECT2D otherwise runs ~0.6us
            # ahead of the ACT unit, and a fast SDMA pickup reads
            # half-written staging (intermittent corruption observed).
            dst1 = out_ext[2 * p + 1 : 2 * p + 2].rearrange("b c h w -> (b c) h w")
            scalar.wait_ge(s_mm, 2 * (c + 1))
            scalar.copy(
                out=obs[1][c][:], in_=banks[c % 4][1][:, 0:rows, :]
            ).then_inc(s_cp2, 1)
            scalar.wait_ge(s_cp2, c + 1)
            scalar.dma_start(
                out=dst1[:, h0 : h0 + rows, :], in_=obs[1][c][:]
            ).then_inc(s_out, 16)
        # Final wait: ALL output DMAs must complete before program end.
        # Skipping this wait measures ~1.9us faster, BUT leaves in-flight
        # DMAs at NEFF teardown -- observed to wedge a core's DMA queue
        # across NEFF reloads into PERSISTENT output corruption (rel err
        # 0.43, sticky across runs). Not worth it.
        sync.wait_ge(s_out, 16 * n_out_dmas)

    return nc


def _prep_inputs(x, K, mm_dt=MM_DT):
    np_dt = mybir.dt.np(mm_dt)
    x = np.ascontiguousarray(np.asarray(x, dtype=np.float32))
    K = np.ascontiguousarray(np.asarray(K, dtype=np.float32))
    xpad = np.pad(x, ((0, 0), (0, 0), (1, 1), (1, 1))).astype(np_dt)
    Wt = K.reshape(KS * KS * C_IN, C_OUT).reshape(C_IN, KS * KS, C_OUT)
    Wrep = np.ascontiguousarray(np.concatenate([Wt, Wt], axis=0)).astype(np_dt)
    shards = xpad.reshape(N_CORES, BPC, C_IN, HP, HP)
    return [{"x": np.ascontiguousarray(shards[i]), "w": Wrep} for i in range(N_CORES)]


def run(x, K, trace=False, mm_dt=MM_DT, njunk=NJUNK):
    nc = build_nc(mm_dt, njunk)
    in_maps = _prep_inputs(x, K, mm_dt)
    res = run_bass_kernel_spmd(nc, in_maps, list(range(N_CORES)), trace=trace)
    out = np.concatenate([res.results[i]["out"] for i in range(N_CORES)], axis=0)
    return out.astype(np.float32), res


def kernel(x, K):
    out, _ = run(x, K, trace=False)
    return out


# revision 41
# speedup vs baseline: 1.0106x; 1.0106x over previous
"""Trainium2 Bass kernel for nn_Conv2d_24833500905755 (3x3 conv, B=32,
C_in=64, C_out=128, 56x56, pad 1, with the reference's mismatched
weight-flatten order).

Math: out[b,co,h,w] = sum_{c,di,dj} xpad[b,c,h+di,w+dj] * Wt[c,di*3+dj,co]
with Wt = K.reshape(576, C_OUT).reshape(C_IN, 9, C_OUT).

Data-parallel: 4 images per NeuronCore, 2 images packed on the
128-partition dim (fp16 matmuls, K=64 contraction per half, concurrent
PE row-group tiles). Raw-bass hand-scheduled engine programs.

Final design (~39.7us measured vs 40.6us session baseline; exec window
sometimes includes ~6.5us of NEFF init -> 45-46us readings, pure
profiler luck):
  - BLOCKLESS: all engine programs emitted into the main BB; cross-
    engine order is fully semaphore-enforced, so nc.Block()'s entry
    branches + exit barrier are skipped.
  - Input DMAs issued first, split over the two HWDGE rings: sync ring
    [p0, p1, p2] (x pair-0 pieces), scalar ring [W, q0, q1, q2]. Ring
    FIFO puts p0/W at the heads; SDMA round-robins between rings at
    packet granularity so both land by ~11.3us. MEASURED DEAD ENDS:
    all-inputs-on-one-ring (8 serial 0.7us desc-gens -> stream stalls,
    +6us); W split into tap pieces across rings (starves behind p1/p2
    in round-robin, W-tail at 13.4us, +4us); tile_position weight
    sharing to halve W (walrus codegen rejects cross-partition LDW);
    p1/p2 via gpsimd SWDGE to unshare early bandwidth (+1us, noisier).
  - Junk warm-up bridge (NJUNK=22 junk MMs, alternating PE row-group
    halves) covers block entry (~7.3us) to W arrival: HAM lifts the PE
    clock 1.2->2.4GHz at a free-running 3.4us window boundary after
    sustained activity; ANY pre-warm PE-idle gap defers the lift ~2us,
    so the bridge must overshoot W slightly.
  - Scalar's out-DMA desc-gen gated on its own copy COMPLETION (s_cp2):
    the sequencer's DIRECT2D otherwise runs 0.6us ahead of the ACT
    unit and a fast SDMA pickup reads half-written staging
    (intermittent 1e-2 corruption observed without the gate).
  - Full final wait on s_out KEPT. Dropping it measures ~1.9us faster
    and usually passes, but in-flight DMAs at NEFF teardown can wedge a
    core's queue across NEFF reloads into PERSISTENT garbage outputs
    (rel err 0.43, sticky until the queue recovers) -- seen when
    different kernel builds run back-to-back. Correctness first.
Fixed costs (verified untouchable): ~6.5us NEFF init (NRT sem-clear
wait + iram loads + barriers), ~0.9us entry preamble, ~6.5us runtime
epilogue (per-engine sem-op storm; NOT in the NEFF engine programs),
~0.7us/DMA HWDGE desc-gen, ~0.8us SDMA first-byte, ~0.4us sem receipt.
Stream itself runs at the fp16 MAC floor (pair of concurrent K=64
row-group matmuls = 448 cols / 2.4GHz =~ 190ns; 126 pairs =~ 24us).
"""

from contextlib import ExitStack

import numpy as np

import concourse.bass as bass
import concourse.mybir as mybir
from concourse.bass_utils import run_bass_kernel_spmd

B, C_IN, C_OUT, H = 32, 64, 128, 56
KS = 3
N_CORES = 8
BPC = B // N_CORES
HP = H + 2
MM_DT = mybir.dt.float16
NJUNK = 22                    # warm-up bridge matmuls (11 concurrent pairs):
                              # bridge ends ~11.4us =~ W + p0 arrival. Too
                              # short leaves a PE-idle gap before W lands,
                              # which resets the HAM warm-up window (~2us
                              # penalty); too long wastes ~0.37us/pair.
JR = 8                        # full-width junk (N=448), ALTERNATING row-group
                              # halves: HAM only registers "busy" (and lifts
                              # the PE clock gate) when both 64-row groups are
                              # streaming, i.e. full-array activity

# per-pair chunk lists: (start_row, n_rows). NOTE: a 4+4 split of the
# final chunk was tried twice (with and without the final s_out wait)
# and loses ~0.5us both ways: scalar's per-chunk copy+gate+desc chain
# (~1.2us) outlasts a 4-row chunk's MM span (~0.86us), so the sub-chunks
# serialize on scalar and the final DMA lands later, not earlier.
CHUNKS = [
    [(i * 8, 8) for i in range(7)],
    [(i * 8, 8) for i in range(7)],
]
NCH = [len(c) for c in CHUNKS]
CHUNK_OF = [(p, ci) for p in range(2) for ci in range(NCH[p])]
NCHT = len(CHUNK_OF)          # 14 global chunks


def build_nc(mm_dt=MM_DT, njunk=NJUNK):
    f32 = mybir.dt.float32
    nc = bass.Bass()
    x_ext = nc.declare_dram_parameter("x", [BPC, C_IN, HP, HP], mm_dt, isOutput=False)
    w_ext = nc.declare_dram_parameter("w", [2 * C_IN, KS * KS, C_OUT], mm_dt, isOutput=False)
    out_ext = nc.declare_dram_parameter("out", [BPC, C_OUT, H, H], mm_dt, isOutput=True)

    n_out_dmas = 2 * NCHT  # halves * chunks

    with ExitStack() as ctx:
        wt = ctx.enter_context(nc.sbuf_tensor("wt", [2 * C_IN, KS * KS, C_OUT], mm_dt))
        xps = [
            ctx.enter_context(nc.sbuf_tensor(f"xp{p}", [2 * C_IN, HP, HP], mm_dt))
            for p in range(2)
        ]
        # obs[half][chunk] - per-chunk fp16 staging (global chunk index)
        obs = [
            [
                ctx.enter_context(
                    nc.sbuf_tensor(
                        f"ob_{h}_{c}", [C_OUT, CHUNKS[p][ci][1], H], mm_dt
                    )
                )
                for c, (p, ci) in enumerate(CHUNK_OF)
            ]
            for h in range(2)
        ]
        actp = ctx.enter_context(nc.sbuf_tensor("actp", [C_OUT, 1], f32))
        # banks[slot][half] - 8 PSUM banks
        banks = [
            [
                ctx.enter_context(
                    nc.psum_tensor(f"ps_{s}_{h}", [C_OUT, 8, H], f32)
                )
                for h in range(2)
            ]
            for s in range(4)
        ]
        s_w = ctx.enter_context(nc.semaphore("s_w"))
        s_x = [ctx.enter_context(nc.semaphore(f"s_x{p}")) for p in range(2)]
        s_mm = ctx.enter_context(nc.semaphore("s_mm"))
        s_cp = ctx.enter_context(nc.semaphore("s_cp"))
        s_cp2 = ctx.enter_context(nc.semaphore("s_cp2"))
        s_out = ctx.enter_context(nc.semaphore("s_out"))

        src0 = x_ext[0:2].rearrange("b c h w -> (b c) h w")
        src1 = x_ext[2:4].rearrange("b c h w -> (b c) h w")
        # Two-ring split, measured best: desc-gen runs 2-wide (one DIRECT2D
        # =~0.7us of NX time per DMA) and W/p0 sit at each ring's head.
        # (All-on-one-ring serializes 8 desc-gens -> pieces land late and
        # the stream stalls; W split across pieces starves behind p1/p2 in
        # the packet round-robin -- both measured slower.)
        # NOTE: keeping p0/p1/p2 on ONE ring is also a CORRECTNESS choice:
        # the s_x thresholds (16/32/48) assume piece completion order,
        # which only same-ring FIFO guarantees. A p0-solo-on-sync variant
        # (to give W full bandwidth after p0) measured no faster AND broke
        # that guarantee.
        nc.sync.dma_start(out=xps[0][:, 0:10, :], in_=src0[:, 0:10, :]).then_inc(s_x[0], 16)
        nc.scalar.dma_start(out=wt[:], in_=w_ext[:]).then_inc(s_w, 16)
        nc.sync.dma_start(out=xps[0][:, 10:34, :], in_=src0[:, 10:34, :]).then_inc(s_x[0], 16)
        nc.scalar.dma_start(out=xps[1][:, 0:10, :], in_=src1[:, 0:10, :]).then_inc(s_x[1], 16)
        nc.sync.dma_start(out=xps[0][:, 34:HP, :], in_=src0[:, 34:HP, :]).then_inc(s_x[0], 16)
        nc.scalar.dma_start(out=xps[1][:, 10:34, :], in_=src1[:, 10:34, :]).then_inc(s_x[1], 16)
        nc.scalar.dma_start(out=xps[1][:, 34:HP, :], in_=src1[:, 34:HP, :]).then_inc(s_x[1], 16)

        # Warm-up bridge: junk matmuls on not-yet-loaded SBUF keep the PE's
        # HAM activity window hot while the input DMAs land. banks[3] is
        # first reused by chunk 3 (start=True clears it), well after these
        # complete.
        for wi in range(njunk):
            h = wi % 2
            c0 = h * C_IN
            nc.tensor.matmul(
                out=banks[3][h][:, 0:JR, :],
                lhsT=wt[c0 : c0 + C_IN, 0, :],
                rhs=xps[0][c0 : c0 + C_IN, 0:JR, 0:H],
                start=True,
                stop=True,
            )

        # Blockless: every engine program is emitted straight into the main
        # BB (like the pre-block DMAs above). Per-engine instruction order =
        # issue order, and cross-engine ordering is already fully semaphore-
        # enforced, so the Block's entry branches and exit barrier (~0.5-1us)
        # are pure overhead.
        sync, scalar, tensor, vector = nc.sync, nc.scalar, nc.tensor, nc.vector

        # scalar: trigger the ACT-table load now; first PSUM copy needs the
        # table at ~14us.
        scalar.copy(out=actp[:], in_=actp[:])

        tensor.wait_ge(s_w, 16)
        for c, (p, ci) in enumerate(CHUNK_OF):
            h0, rows = CHUNKS[p][ci]
            if ci == 0:
                tensor.wait_ge(s_x[p], 16)  # rows [0,10)
            elif ci == 1:
                tensor.wait_ge(s_x[p], 32)  # rows [10,34)
            elif ci == 4:
                tensor.wait_ge(s_x[p], 48)  # rows [34,58)
            if c >= 4:
                # WAR: bank slot c%4 last used by chunk c-4
                tensor.wait_ge(s_cp, c - 3)
                tensor.wait_ge(s_cp2, c - 3)
            for k in range(KS * KS):
                di, dj = divmod(k, KS)
                last = k == KS * KS - 1
                for half in range(2):
                    c0 = half * C_IN
                    mm = tensor.matmul(
                        out=banks[c % 4][half][:, 0:rows, :],
                        lhsT=wt[c0 : c0 + C_IN, k, :],
                        rhs=xps[p][
                            c0 : c0 + C_IN,
                            h0 + di : h0 + di + rows,
                            dj : dj + H,
                        ],
                        start=(k == 0),
                        stop=last,
                    )
                    if last and half == 1:
                        mm.then_inc(s_mm, 2)

            # vector: half-0 PSUM->SBUF fp32->fp16 copy
            vector.wait_ge(s_mm, 2 * (c + 1))
            vector.tensor_copy(
                out=obs[0][c][:],
                in_=banks[c % 4][0][:, 0:rows, :],
            ).then_inc(s_cp, 1)

            # sync: half-0 output DMA (gated on vector copy completion)
            dst0 = out_ext[2 * p : 2 * p + 1].rearrange("b c h w -> (b c) h w")
            sync.wait_ge(s_cp, c + 1)
            sync.dma_start(
                out=dst0[:, h0 : h0 + rows, :], in_=obs[0][c][:]
            ).then_inc(s_out, 16)

            # scalar: half-1 copy + output DMA. Desc-gen gated on copy
            # COMPLETION: the sequencer's DIRECT2D otherwise runs ~0.6us
            # ahead of the ACT unit, and a fast SDMA pickup reads
            # half-written staging (intermittent corruption observed).
            dst1 = out_ext[2 * p + 1 : 2 * p + 2].rearrange("b c h w -> (b c) h w")
            scalar.wait_ge(s_mm, 2 * (c + 1))
            scalar.copy(
                out=obs[1][c][:], in_=banks[c % 4][1][:, 0:rows, :]
            ).then_inc(s_cp2, 1)
            scalar.wait_ge(s_cp2, c + 1)
            scalar.dma_start(
                out=dst1[:, h0 : h0 + rows, :], in_=obs[1][c][:]
            ).then_inc(s_out, 16)
        # Final wait: ALL output DMAs must complete before program end.
        # Skipping this wait measures ~1.9us faster, BUT leaves in-flight
        # DMAs at NEFF teardown -- observed to wedge a core's DMA queue
        # across NEFF reloads into PERSISTENT output corruption (rel err
        # 0.43, sticky across runs). Not worth it.
        sync.wait_ge(s_out, 16 * n_out_dmas)

    return nc


def _prep_inputs(x, K, mm_dt=MM_DT):
    np_dt = mybir.dt.np(mm_dt)
    x = np.ascontiguousarray(np.asarray(x, dtype=np.float32))
    K = np.ascontiguousarray(np.asarray(K, dtype=np.float32))
    xpad = np.pad(x, ((0, 0), (0, 0), (1, 1), (1, 1))).astype(np_dt)
    Wt = K.reshape(KS * KS * C_IN, C_OUT).reshape(C_IN, KS * KS, C_OUT)
    Wrep = np.ascontiguousarray(np.concatenate([Wt, Wt], axis=0)).astype(np_dt)
    shards = xpad.reshape(N_CORES, BPC, C_IN, HP, HP)
    return [{"x": np.ascontiguousarray(shards[i]), "w": Wrep} for i in range(N_CORES)]


def run(x, K, trace=False, mm_dt=MM_DT, njunk=NJUNK):
    nc = build_nc(mm_dt, njunk)
    in_maps = _prep_inputs(x, K, mm_dt)
    res = run_bass_kernel_spmd(nc, in_maps, list(range(N_CORES)), trace=trace)
    out = np.concatenate([res.results[i]["out"] for i in range(N_CORES)], axis=0)
    return out.astype(np.float32), res


def kernel(x, K):
    out, _ = run(x, K, trace=False)
    return out
